# revision 1
# baseline (speedup 1.0000x reference)
"""Trainium2 Bass kernel for nn_MetaKRec (LightGCN over 3 graphs + attention combine).

Reference:
    for each of 3 graphs: h = emb_table[x]; 3x LGConv (sym-normalized SpMM)
    emb = stack(h_g) [N,3,D]; score = (emb@W)@a -> softmax over graphs
    node = sum(w_g * emb_g); out[b] = node[user_b] . node[item_b]

Device algorithm (8-core SPMD):
  Normalization folded into per-node scales: u = dinv*h; per layer
  s[v] = sum_{e:dst=v} u[src_e]; u' = dinv^2*s (inner) / dinv*s (last).

  Nodes dst-sharded 8 ways (12500/core, padded to 12544 = 128*98). Per core,
  edges targeting its shard are sorted by destination window (128 dsts).
  Per 128-edge tile:
    - [P,1] indirect DMA gathers the 128 source rows u[src] from the
      replicated full-u table in HBM (one row per partition),
    - a DVE is_equal against a constant iota plane builds the one-hot
      scatter matrix S[e, j] = (dst_rel_e == j),
    - PE matmul  psum[128 dst, 64] += S.T @ msg  accumulates the segment sum
      window-aligned; flushed to the SBUF shard accumulator per window.
  Scale by dinv^2, AllGather the 3.2MB shard to rebuild u. All float math on
  device; host does only integer bucketing/sorting/layout.
"""

import os
import sys

for _p in ("/opt/trn_rl_repo",):
    if _p not in sys.path and os.path.isdir(_p):
        sys.path.insert(0, _p)

import numpy as np

import concourse.bass as bass
import concourse.bacc as bacc
import concourse.mybir as mybir
import concourse.tile as tile
from concourse import bass_utils
from concourse.bass import IndirectOffsetOnAxis

F32 = mybir.dt.float32
BF16 = mybir.dt.bfloat16
I32 = mybir.dt.int32

NCORES = 8
G = 3
LAYERS = 3
P = 128


def _pack_core_graph(srcp, dst_local, shard, cs):
    """Sort edges by dst window, pad each window's edges to a multiple of P.

    Returns (src_ids[E_pad], dst_rel[E_pad] fp32, tiles_per_window[cs]).
    Pad slots: src 0, dst_rel -1 (one-hot all-zero -> contributes nothing).
    """
    win = dst_local // P
    order = np.argsort(win, kind="stable")
    srcp, dst_local, win = srcp[order], dst_local[order], win[order]
    counts = np.bincount(win, minlength=cs)
    tiles = np.maximum(1, (counts + P - 1) // P)
    src_out, rel_out = [], []
    pos = 0
    for w in range(cs):
        c = int(counts[w])
        t = int(tiles[w])
        pad = t * P - c
        src_out.append(srcp[pos:pos + c])
        src_out.append(np.zeros(pad, dtype=np.int64))
        rel_out.append((dst_local[pos:pos + c] - P * w).astype(np.float32))
        rel_out.append(np.full(pad, -1.0, dtype=np.float32))
        pos += c
    return (np.concatenate(src_out), np.concatenate(rel_out), tiles)


def preprocess(N, D, B, x, edge_indices, emb_table, W, a, user, item):
    """Host-side integer/layout preprocessing. Returns (in_maps, static, pos_of_b)."""
    SHARD = N // NCORES
    CS = (SHARD + P - 1) // P
    SPAD = P * CS
    NPAD = NCORES * SPAD

    h0 = np.asarray(emb_table, dtype=np.float32)[np.asarray(x, dtype=np.int64)]

    def to_pad(v):
        return SPAD * (v // SHARD) + (v % SHARD)

    degs = [np.bincount(np.asarray(ei[1], dtype=np.int64), minlength=N)
            .astype(np.float32) for ei in edge_indices]

    # per (core, graph) packed edge arrays
    packed = [[None] * G for _ in range(NCORES)]
    for g, ei in enumerate(edge_indices):
        src = np.asarray(ei[0], dtype=np.int64)
        dst = np.asarray(ei[1], dtype=np.int64)
        srcp = to_pad(src)
        core_of = dst // SHARD
        for r in range(NCORES):
            m = core_of == r
            packed[r][g] = _pack_core_graph(srcp[m], dst[m] - r * SHARD, SHARD, CS)

    # unify tiles-per-window across cores (SPMD: one program)
    tiles_per_win = [
        np.max([packed[r][g][2] for r in range(NCORES)], axis=0) for g in range(G)
    ]
    # re-pad each core's arrays to the unified widths
    for g in range(G):
        tw = tiles_per_win[g]
        for r in range(NCORES):
            s_r, rel_r, t_r = packed[r][g]
            src_out, rel_out = [], []
            pos = 0
            for w in range(CS):
                n_old = int(t_r[w]) * P
                n_new = int(tw[w]) * P
                src_out.append(s_r[pos:pos + n_old])
                rel_out.append(rel_r[pos:pos + n_old])
                if n_new > n_old:
                    src_out.append(np.zeros(n_new - n_old, dtype=np.int64))
                    rel_out.append(np.full(n_new - n_old, -1.0, dtype=np.float32))
                pos += n_old
            packed[r][g] = (np.concatenate(src_out), np.concatenate(rel_out), tw)

    T_tot = [int(tiles_per_win[g].sum()) for g in range(G)]

    # readout positions: pad B to multiple of P
    user = np.asarray(user, dtype=np.int64)
    item = np.asarray(item, dtype=np.int64)
    PB = ((B + P - 1) // P) * P
    up = np.zeros(PB, dtype=np.int64)
    ip = np.zeros(PB, dtype=np.int64)
    up[:B] = to_pad(user)
    ip[:B] = to_pad(item)
    pos_of_b = np.arange(B)

    in_maps = []
    for r in range(NCORES):
        m = {}
        sh = np.zeros((SPAD, D), dtype=np.float32)
        sh[:SHARD] = h0[r * SHARD:(r + 1) * SHARD]
        m["h0_shard"] = sh
        dg = np.zeros((G, P, CS), dtype=np.float32)
        for g in range(G):
            pad = np.zeros(SPAD, dtype=np.float32)
            pad[:SHARD] = degs[g][r * SHARD:(r + 1) * SHARD]
            dg[g] = pad.reshape(CS, P).T
        m["deg"] = dg
        for g in range(G):
            s_r, rel_r, _ = packed[r][g]
            # tile t occupies column t: [P, T_tot]
            m[f"srcids{g}"] = s_r.reshape(T_tot[g], P).T.astype(np.int32).copy()
            m[f"dstrel{g}"] = rel_r.reshape(T_tot[g], P).T.astype(np.float32).copy()
        m["W"] = np.asarray(W, dtype=np.float32)
        m["a_vec"] = np.asarray(a, dtype=np.float32).reshape(D, 1)
        m["uids"] = up.reshape(PB // P, P).T.astype(np.int32).copy()
        m["iids"] = ip.reshape(PB // P, P).T.astype(np.int32).copy()
        iota = np.tile(np.arange(P, dtype=np.float32), (P, 1))
        m["iotaF"] = iota
        in_maps.append(m)

    static = dict(N=N, D=D, B=B, SHARD=SHARD, CS=CS, SPAD=SPAD, NPAD=NPAD,
                  PB=PB, tiles_per_win=tiles_per_win, T_tot=T_tot)
    return in_maps, static, pos_of_b


def build_program(st):
    D, CS, SPAD, NPAD, PB = st["D"], st["CS"], st["SPAD"], st["NPAD"], st["PB"]
    tiles_per_win, T_tot = st["tiles_per_win"], st["T_tot"]

    nc = bacc.Bacc("TRN2", target_bir_lowering=False, debug=False,
                   num_devices=NCORES)

    h0_shard = nc.dram_tensor("h0_shard", [SPAD, D], F32, kind="ExternalInput")
    deg_in = nc.dram_tensor("deg", [G, P, CS], F32, kind="ExternalInput")
    srcids = [nc.dram_tensor(f"srcids{g}", [P, T_tot[g]], I32, kind="ExternalInput")
              for g in range(G)]
    dstrel = [nc.dram_tensor(f"dstrel{g}", [P, T_tot[g]], F32, kind="ExternalInput")
              for g in range(G)]
    W_in = nc.dram_tensor("W", [D, D], F32, kind="ExternalInput")
    a_in = nc.dram_tensor("a_vec", [D, 1], F32, kind="ExternalInput")
    uids_in = nc.dram_tensor("uids", [P, PB // P], I32, kind="ExternalInput")
    iids_in = nc.dram_tensor("iids", [P, PB // P], I32, kind="ExternalInput")
    iota_in = nc.dram_tensor("iotaF", [P, P], F32, kind="ExternalInput")
    out_dots = nc.dram_tensor("out_dots", [P, PB // P], F32, kind="ExternalOutput")

    rg = [list(range(NCORES))]

    with tile.TileContext(nc) as tc:
        with (
            tc.tile_pool(name="dram", bufs=1, space="DRAM") as dpool,
            tc.tile_pool(name="const", bufs=1) as cpool,
            tc.tile_pool(name="shard", bufs=3) as shpool,
            tc.tile_pool(name="msg", bufs=3) as mpool,
            tc.tile_pool(name="oneh", bufs=3) as opool,
            tc.tile_pool(name="ps", bufs=2, space="PSUM") as ppool,
        ):
            U = [[dpool.tile([NPAD, 2 * D], BF16, addr_space="Shared", tag=f"U{g}_{i}", name=f"U{g}_{i}")
                  for i in range(LAYERS)] for g in range(G)]
            ag_in = [dpool.tile([SPAD, 2 * D], BF16, tag=f"agin{g}", name=f"agin{g}") for g in range(G)]
            emb_d = [dpool.tile([SPAD, D], F32, tag=f"emb{g}", name=f"embd{g}") for g in range(G)]
            node_full = dpool.tile([NPAD, D], F32, addr_space="Shared", tag="nodef")
            node_in = dpool.tile([SPAD, D], F32, tag="nodein")

            def sh3(dram2d):
                return dram2d.rearrange("(c p) d -> p c d", p=P)

            def pack_and_send(ut, g):
                pk = shpool.tile([P, CS, 2 * D], BF16, tag="pk", bufs=2)
                nc.vector.tensor_copy(pk[:, :, 0:D], ut[:])
                nc.vector.tensor_tensor(out=pk[:, :, D:2 * D], in0=ut[:],
                                        in1=pk[:, :, 0:D],
                                        op=mybir.AluOpType.subtract)
                nc.sync.dma_start(sh3(ag_in[g][:]), pk[:])

            iotaF = cpool.tile([P, P], F32, tag="iotaF")
            nc.sync.dma_start(iotaF[:], iota_in.ap())

            # dinv grids
            dinv = [cpool.tile([P, CS], F32, tag=f"dinv{g}", name=f"dinv{g}") for g in range(G)]
            dinv2 = [cpool.tile([P, CS], F32, tag=f"dinv2{g}", name=f"dinv2{g}") for g in range(G)]
            for g in range(G):
                dt_ = cpool.tile([P, CS], F32, tag="degtmp")
                nc.sync.dma_start(dt_[:], deg_in[g])
                mx = cpool.tile([P, CS], F32, tag="degmax")
                nc.vector.tensor_scalar(out=mx[:], in0=dt_[:], scalar1=1e-12,
                                        scalar2=None, op0=mybir.AluOpType.max)
                sq = cpool.tile([P, CS], F32, tag="degsq")
                nc.scalar.activation(sq[:], mx[:], mybir.ActivationFunctionType.Sqrt)
                rc = cpool.tile([P, CS], F32, tag="degrc")
                nc.vector.reciprocal(rc[:], sq[:])
                mask = cpool.tile([P, CS], F32, tag="degmask")
                nc.vector.tensor_scalar(out=mask[:], in0=dt_[:], scalar1=0.0,
                                        scalar2=None, op0=mybir.AluOpType.is_gt)
                nc.vector.tensor_tensor(out=dinv[g][:], in0=rc[:], in1=mask[:],
                                        op=mybir.AluOpType.mult)
                nc.vector.tensor_tensor(out=dinv2[g][:], in0=dinv[g][:],
                                        in1=dinv[g][:], op=mybir.AluOpType.mult)

            Tmax_tot = max(T_tot)

            # u0 prep
            for g in range(G):
                h0t = shpool.tile([P, CS, D], F32, tag="big3")
                nc.sync.dma_start(h0t[:], sh3(h0_shard.ap()))
                u0t = shpool.tile([P, CS, D], F32, tag="big3")
                nc.vector.tensor_tensor(
                    out=u0t[:], in0=h0t[:],
                    in1=dinv[g][:].rearrange("p c -> p c ()").to_broadcast([P, CS, D]),
                    op=mybir.AluOpType.mult)
                pack_and_send(u0t, g)
                nc.gpsimd.collective_compute(
                    "AllGather", mybir.AluOpType.bypass, replica_groups=rg,
                    ins=[ag_in[g].opt()], outs=[U[g][0].opt()])

            # layers
            pending_ag = []
            for layer in range(LAYERS):
                for g in range(G):
                    ubuf = U[g][layer]
                    tw = tiles_per_win[g]
                    src_g = cpool.tile([P, Tmax_tot], I32, tag="src_sb", bufs=2)
                    nc.sync.dma_start(src_g[:, :T_tot[g]], srcids[g].ap())
                    rel_g = cpool.tile([P, Tmax_tot], F32, tag="rel_sb", bufs=2)
                    nc.sync.dma_start(rel_g[:, :T_tot[g]], dstrel[g].ap())
                    Tmax = int(max(int(tiles_per_win[gg].max()) for gg in range(G)))
                    s_sh = shpool.tile([P, CS, D], F32, tag="big3")
                    t0 = 0
                    for w in range(CS):
                        T = int(tw[w])
                        # one-hot for this window: [P, T, P]
                        oneh = opool.tile([P, Tmax, P], BF16, tag="oneh")
                        nc.vector.tensor_tensor(
                            out=oneh[:, :T, :],
                            in0=rel_g[:, t0:t0 + T]
                                .rearrange("p t -> p t ()").to_broadcast([P, T, P]),
                            in1=iotaF[:].rearrange("p j -> p () j")
                                .to_broadcast([P, T, P]),
                            op=mybir.AluOpType.is_equal)
                        psum = ppool.tile([P, 2 * D], F32, tag="acc_ps", bufs=3)
                        wt = mpool.tile([P, Tmax, 2 * D], BF16, tag="msg")
                        for i in range(T):
                            nc.gpsimd.indirect_dma_start(
                                out=wt[:, i, :], out_offset=None, in_=ubuf[:],
                                in_offset=IndirectOffsetOnAxis(
                                    ap=src_g[:, t0 + i:t0 + i + 1], axis=0))
                            nc.tensor.matmul(psum[:], lhsT=oneh[:, i, :],
                                             rhs=wt[:, i, :], start=(i == 0),
                                             stop=(i == T - 1))
                        nc.scalar.activation(s_sh[:, w, :], psum[:, 0:D],
                                             mybir.ActivationFunctionType.Copy)
                        nc.vector.tensor_tensor(out=s_sh[:, w, :],
                                                in0=s_sh[:, w, :],
                                                in1=psum[:, D:2 * D],
                                                op=mybir.AluOpType.add)
                        t0 += T
                    if pending_ag:
                        pending_ag.pop(0)()
                    # scale
                    ut = shpool.tile([P, CS, D], F32, tag="big3")
                    fac = dinv2[g] if layer < LAYERS - 1 else dinv[g]
                    nc.vector.tensor_tensor(
                        out=ut[:], in0=s_sh[:],
                        in1=fac[:].rearrange("p c -> p c ()").to_broadcast([P, CS, D]),
                        op=mybir.AluOpType.mult)
                    if layer < LAYERS - 1:
                        pack_and_send(ut, g)

                        def _ag(gg=g, ll=layer):
                            nc.gpsimd.collective_compute(
                                "AllGather", mybir.AluOpType.bypass,
                                replica_groups=rg,
                                ins=[ag_in[gg].opt()], outs=[U[gg][ll + 1].opt()])
                        pending_ag.append(_ag)
                    else:
                        nc.sync.dma_start(sh3(emb_d[g][:]), ut[:])

            for _f in pending_ag:
                _f()
            pending_ag = []

            # combine: wa = W @ a, broadcast to [P, D]
            wT = cpool.tile([D, D], F32, tag="wT")
            nc.gpsimd.dma_start(wT[:], W_in.ap().rearrange("d e -> e d"))
            a_t = cpool.tile([D, 1], F32, tag="a_t")
            nc.sync.dma_start(a_t[:], a_in.ap())
            wa_ps = ppool.tile([1, D], F32, tag="wa_ps", bufs=1)
            nc.tensor.matmul(wa_ps[:], a_t[:], wT[:])
            wa_row = cpool.tile([1, D], F32, tag="wa_row")
            nc.vector.tensor_copy(wa_row[:], wa_ps[:])
            ones_t = cpool.tile([1, P], F32, tag="ones")
            nc.vector.memset(ones_t[:], 1.0)
            wab_ps = ppool.tile([P, D], F32, tag="wab_ps", bufs=1)
            nc.tensor.matmul(wab_ps[:], ones_t[:], wa_row[:])
            wa_bc = cpool.tile([P, D], F32, tag="wa_bc")
            nc.vector.tensor_copy(wa_bc[:], wab_ps[:])

            sc = [cpool.tile([P, CS], F32, tag=f"sc{g}", name=f"sc{g}") for g in range(G)]
            for g in range(G):
                e_t = shpool.tile([P, CS, D], F32, tag="big3")
                nc.sync.dma_start(e_t[:], sh3(emb_d[g][:]))
                tmp = shpool.tile([P, CS, D], F32, tag="big3")
                nc.vector.tensor_tensor(
                    out=tmp[:], in0=e_t[:],
                    in1=wa_bc[:].rearrange("p d -> p () d").to_broadcast([P, CS, D]),
                    op=mybir.AluOpType.mult)
                nc.vector.tensor_reduce(out=sc[g][:], in_=tmp[:],
                                        axis=mybir.AxisListType.X,
                                        op=mybir.AluOpType.add)
            mxs = cpool.tile([P, CS], F32, tag="smax")
            nc.vector.tensor_tensor(out=mxs[:], in0=sc[0][:], in1=sc[1][:],
                                    op=mybir.AluOpType.max)
            nc.vector.tensor_tensor(out=mxs[:], in0=mxs[:], in1=sc[2][:],
                                    op=mybir.AluOpType.max)
            ex = [cpool.tile([P, CS], F32, tag=f"ex{g}", name=f"ex{g}") for g in range(G)]
            for g in range(G):
                df = cpool.tile([P, CS], F32, tag="sdiff")
                nc.vector.tensor_tensor(out=df[:], in0=sc[g][:], in1=mxs[:],
                                        op=mybir.AluOpType.subtract)
                nc.scalar.activation(ex[g][:], df[:], mybir.ActivationFunctionType.Exp)
            zs = cpool.tile([P, CS], F32, tag="zsum")
            nc.vector.tensor_tensor(out=zs[:], in0=ex[0][:], in1=ex[1][:],
                                    op=mybir.AluOpType.add)
            nc.vector.tensor_tensor(out=zs[:], in0=zs[:], in1=ex[2][:],
                                    op=mybir.AluOpType.add)
            rz = cpool.tile([P, CS], F32, tag="rz")
            nc.vector.reciprocal(rz[:], zs[:])

            node_t = shpool.tile([P, CS, D], F32, tag="node_t", bufs=1)
            for g in range(G):
                wg = cpool.tile([P, CS], F32, tag="wg")
                nc.vector.tensor_tensor(out=wg[:], in0=ex[g][:], in1=rz[:],
                                        op=mybir.AluOpType.mult)
                e_t = shpool.tile([P, CS, D], F32, tag="big3")
                nc.sync.dma_start(e_t[:], sh3(emb_d[g][:]))
                if g == 0:
                    nc.vector.tensor_tensor(
                        out=node_t[:], in0=e_t[:],
                        in1=wg[:].rearrange("p c -> p c ()").to_broadcast([P, CS, D]),
                        op=mybir.AluOpType.mult)
                else:
                    tmp = shpool.tile([P, CS, D], F32, tag="big3")
                    nc.vector.tensor_tensor(
                        out=tmp[:], in0=e_t[:],
                        in1=wg[:].rearrange("p c -> p c ()").to_broadcast([P, CS, D]),
                        op=mybir.AluOpType.mult)
                    nc.vector.tensor_tensor(out=node_t[:], in0=node_t[:],
                                            in1=tmp[:], op=mybir.AluOpType.add)

            nc.sync.dma_start(sh3(node_in[:]), node_t[:])
            nc.gpsimd.collective_compute(
                "AllGather", mybir.AluOpType.bypass, replica_groups=rg,
                ins=[node_in.opt()], outs=[node_full.opt()])

            # readout
            u_sb = cpool.tile([P, PB // P], I32, tag="u_sb")
            i_sb = cpool.tile([P, PB // P], I32, tag="i_sb")
            nc.sync.dma_start(u_sb[:], uids_in.ap())
            nc.sync.dma_start(i_sb[:], iids_in.ap())
            dots = cpool.tile([P, PB // P], F32, tag="dots")
            for t in range(PB // P):
                ur = mpool.tile([P, D], F32, tag="ur")
                nc.gpsimd.indirect_dma_start(
                    out=ur[:], out_offset=None, in_=node_full[:],
                    in_offset=IndirectOffsetOnAxis(ap=u_sb[:, t:t + 1], axis=0))
                ir = mpool.tile([P, D], F32, tag="ir")
                nc.gpsimd.indirect_dma_start(
                    out=ir[:], out_offset=None, in_=node_full[:],
                    in_offset=IndirectOffsetOnAxis(ap=i_sb[:, t:t + 1], axis=0))
                pr = mpool.tile([P, D], F32, tag="pr")
                nc.vector.tensor_tensor(out=pr[:], in0=ur[:], in1=ir[:],
                                        op=mybir.AluOpType.mult)
                nc.vector.tensor_reduce(out=dots[:, t:t + 1], in_=pr[:],
                                        axis=mybir.AxisListType.X,
                                        op=mybir.AluOpType.add)
            nc.sync.dma_start(out_dots.ap(), dots[:])

    nc.compile()
    return nc


def kernel(user, item, x, edge_index_0, edge_index_1, edge_index_2,
           emb_table, W, a, _run_kwargs=None, _return_res=False,
           _shapes=None):
    N, D, B = 100000, 64, 4096
    if _shapes is not None:
        N, D, B = _shapes
    in_maps, st, pos_of_b = preprocess(
        N, D, B, x, [edge_index_0, edge_index_1, edge_index_2],
        emb_table, W, a, user, item)
    nc = build_program(st)
    res = bass_utils.run_bass_kernel_spmd(
        nc, in_maps, core_ids=list(range(NCORES)), **(_run_kwargs or {}))
    od = np.asarray(res.results[0]["out_dots"])  # [P, PB/P], pos k = [k%P, k//P]
    flat = od.T.reshape(-1)
    out = flat[pos_of_b].astype(np.float32)
    if _return_res:
        return out, res
    return out

